# revision 7
# baseline (speedup 1.0000x reference)
"""Trainium2 Bass kernel for the hardest-positive triplet-softplus loss.

Key observation: the reference builds the full 4096x4096 distance matrix but
only ever *uses* same-label entries (hardest-positive mining per row).  With
C=128 classes over B=4096 rows, each class has ~32 members.  Sorting rows by
label on the host makes every row's positives live in a small contiguous band
of the sorted order, so each 128-row block only needs a 256-column Gram block
instead of 4096 columns: 16x less matmul work and ~8x less HBM traffic.

Strategy (8 NeuronCores, data-parallel over sorted row windows):
  - Host sorts rows by label (stable).  Each core owns 4 windows of 128
    consecutive sorted rows.  For window w starting at sorted position `base`,
    every positive of every row in the window lies within sorted positions
    [base-64, base+192)  (class size <= ~51 here; asserted).
  - Device computes, per window, the [128 x 256] block
        S[p, j] = dot(x_row(base+p), x_col(base-64+j)) + 256 - sq_col/2
    via 4 accumulating fp8-e3m4 matmuls (K=512) plus one K=1 fp16 matmul that
    broadcasts the per-column  256 - sq/2  term.  The lhsT operand is the
    center 128 columns of the same SBUF tile -- no separate lhs upload.
    Everything is DMA-bound, so inputs are fp8 where precision allows.
  - DVE adds a host-built additive byte mask (0 for positives, -28672 for
    self / other-label / out-of-range, stored fp8-e5m2) and row-max-reduces:
        v[p] = max_j (S[p, j] + m[p, j])
    The winner value encodes the hardest-positive distance:
        d2_ap = sq_row + 512 - 2*v        (recovered on host)
  - Everything else is exact host numpy: d_an from the raw fp32 batch,
    softplus tail, valid mask / count, final mean.  Device output is just
    [128, 4] fp32 row-max values per core.
  - DMA triggers are spread across the sync/scalar/gpsimd/vector queues so
    descriptor generation for the 12 input DMAs runs in parallel.
"""

import os
import sys

import numpy as np

for _p in ("/opt/trn_rl_repo", "/root/.axon_site/_ro/trn_rl_repo"):
    if os.path.isdir(_p) and _p not in sys.path:
        sys.path.append(_p)

import ml_dtypes  # noqa: E402

import concourse.bass as bass  # noqa: E402
import concourse.bacc as bacc  # noqa: E402
import concourse.tile as tile  # noqa: E402
from concourse import mybir  # noqa: E402
from concourse import bass_utils  # noqa: E402

B = 4096
DIM = 512
C = 128
TEMP = 0.05
NCORES = 8
NW = 4            # windows of 128 sorted rows per core
W = 256           # columns per window (center 128 + 64 pad each side)
NK = DIM // 128   # 4 contraction tiles
LHS0 = 64         # offset of the window's own rows inside the 256 columns
NEG = -28672.0    # mask value for non-positive columns (exact in fp8 e5m2)

F32 = mybir.dt.float32
F16 = mybir.dt.float16
E3M4 = mybir.dt.float8e3
E5M2 = mybir.dt.float8e5
ALU = mybir.AluOpType
AX = mybir.AxisListType

_NC_CACHE = None


def _build_nc():
    nc = bacc.Bacc(
        "TRN2",
        target_bir_lowering=False,
        debug=False,
        enable_asserts=False,
    )

    xw_d = nc.dram_tensor("xw", [NW, 128, NK, W], E3M4, kind="ExternalInput").ap()
    mk_d = nc.dram_tensor("mk", [NW, 128, W], E5M2, kind="ExternalInput").ap()
    sq_d = nc.dram_tensor("sqw", [NW, 1, W], F16, kind="ExternalInput").ap()
    out_d = nc.dram_tensor("out", [128, NW], F32, kind="ExternalOutput").ap()

    with tile.TileContext(nc) as tc:
        with (
            tc.tile_pool(name="big", bufs=1) as big,
            tc.tile_pool(name="ps", bufs=4, space="PSUM") as pp,
            tc.tile_pool(name="sm", bufs=1) as sm,
        ):
            engines = [nc.sync, nc.scalar, nc.gpsimd]

            ones = sm.tile([1, 128], F16, tag="ones")
            nc.gpsimd.memset(ones[:], 1.0)

            # big x chunks first, one per queue, so the four descriptor
            # generations run concurrently; small mask/sq DMAs follow.
            xt, mt, st = [], [], []
            for w in range(NW):
                t = big.tile([128, NK, W], E3M4, tag=f"x{w}", name=f"x{w}")
                engines[w % 3].dma_start(t[:], xw_d[w])
                xt.append(t)
            for w in range(NW):
                t = sm.tile([128, W], E5M2, tag=f"m{w}", name=f"m{w}")
                engines[(w + 1) % 3].dma_start(t[:], mk_d[w])
                mt.append(t)
                t = sm.tile([1, W], F16, tag=f"s{w}", name=f"s{w}")
                engines[(w + 2) % 3].dma_start(t[:], sq_d[w])
                st.append(t)

            outt = sm.tile([128, NW], F32, tag="outt")
            junk = sm.tile([128, W], F32, tag="junk")

            for w in range(NW):
                pt = pp.tile([128, W], F32, tag="acc", name="acc")
                for k in range(NK):
                    nc.tensor.matmul(
                        pt[:],
                        xt[w][:, k, LHS0:LHS0 + 128],
                        xt[w][:, k, :],
                        start=(k == 0),
                        stop=False,
                    )
                # per-column (256 - sq/2) broadcast via K=1 matmul
                nc.tensor.matmul(
                    pt[:], ones[:], st[w][:], start=False, stop=True)
                # mask add + row max (tensor_tensor_reduce would fuse these
                # but wedges TRN2 hardware)
                nc.vector.tensor_tensor(
                    junk[:], pt[:], mt[w][:], op=ALU.add)
                nc.vector.tensor_reduce(
                    outt[:, w:w + 1], junk[:], axis=AX.X, op=ALU.max)

            nc.sync.dma_start(out_d[:], outt[:])

    nc.compile()
    return nc


def get_nc():
    global _NC_CACHE
    if _NC_CACHE is None:
        _NC_CACHE = _build_nc()
    return _NC_CACHE


def _prep_inputs(batch, labels, anchors=None, negatives=None):
    """Host-side prep: per-core window tensors + (order, sqs) for unshard."""
    batch = np.ascontiguousarray(np.asarray(batch), dtype=np.float32)
    labels = np.asarray(labels).astype(np.int64)

    order = np.argsort(labels, kind="stable").astype(np.int64)
    slab = labels[order]
    xs = batch[order]
    sqs = np.einsum("ij,ij->i", xs, xs, dtype=np.float64)

    xsT = np.ascontiguousarray(xs.T.astype(ml_dtypes.float8_e3m4))   # [DIM, B]
    sqrow = (256.0 - sqs / 2.0).astype(np.float16)                   # [B]

    # containment: every row's class fits in its window's 256 columns
    starts = np.searchsorted(slab, slab, side="left")
    ends = np.searchsorted(slab, slab, side="right")

    in_maps = []
    for c in range(NCORES):
        xw = np.empty((NW, 128, NK, W), ml_dtypes.float8_e3m4)
        mk = np.empty((NW, 128, W), ml_dtypes.float8_e5m2)
        sqw = np.empty((NW, 1, W), np.float16)
        for wl in range(NW):
            base = (c * NW + wl) * 128
            assert starts[base] >= base - LHS0, "class overflows window left pad"
            assert ends[base + 127] <= base + (W - LHS0), (
                "class overflows window right pad")
            colpos = base - LHS0 + np.arange(W)
            validc = (colpos >= 0) & (colpos < B)
            cp = np.clip(colpos, 0, B - 1)
            xw[wl] = xsT[:, cp].reshape(NK, 128, W).transpose(1, 0, 2)
            sqw[wl, 0] = sqrow[cp]
            rowpos = base + np.arange(128)
            ok = (validc[None, :]
                  & (slab[cp][None, :] == slab[rowpos][:, None])
                  & (colpos[None, :] != rowpos[:, None]))
            mk[wl] = np.where(ok, 0.0, NEG).astype(ml_dtypes.float8_e5m2)
        in_maps.append({"xw": xw, "mk": mk, "sqw": sqw})
    return in_maps, order, sqs


def kernel(batch, labels, anchors=None, negatives=None, **_kwargs):
    batch = np.ascontiguousarray(np.asarray(batch), dtype=np.float32)
    labels_np = np.asarray(labels).astype(np.int64)
    negatives_np = np.asarray(negatives).astype(np.int64)

    in_maps, order, sqs = _prep_inputs(batch, labels_np)
    nc = get_nc()
    res = bass_utils.run_bass_kernel_spmd(nc, in_maps, core_ids=list(range(NCORES)))

    v = np.stack([np.asarray(r["out"], dtype=np.float64) for r in res.results])
    vsorted = v.transpose(0, 2, 1).reshape(B)     # [core, w, p] -> sorted pos
    d2ap_sorted = sqs + 512.0 - 2.0 * vsorted
    d2_ap = np.empty(B, dtype=np.float64)
    d2_ap[order] = d2ap_sorted
    d_ap = np.sqrt(np.maximum(d2_ap, 1e-12))

    diff = batch.astype(np.float64) - batch[negatives_np].astype(np.float64)
    d_an = np.sqrt(np.maximum(np.einsum("ij,ij->i", diff, diff), 1e-12))

    z = (d_ap - d_an) / (2.0 * TEMP)
    per = np.logaddexp(0.0, z)

    hist = np.bincount(labels_np, minlength=C)
    valid = (hist[labels_np] - 1) > 1
    count = float(valid.sum())
    loss = float((per * valid.astype(np.float64)).sum() / count)
    return np.array([loss], dtype=np.float32)


# revision 8
# speedup vs baseline: 1.2017x; 1.2017x over previous
"""Trainium2 Bass kernel for the hardest-positive triplet-softplus loss.

Key observation: the reference builds the full 4096x4096 distance matrix but
only ever *uses* same-label entries (hardest-positive mining per row).  With
C=128 classes over B=4096 rows, each class has ~32 members.  Sorting rows by
label on the host makes every row's positives live in a small contiguous band
of the sorted order, so each 128-row block only needs a 256-column Gram block
instead of 4096 columns: 16x less matmul work and ~8x less HBM traffic.

Strategy (8 NeuronCores, data-parallel over sorted row windows):
  - Host sorts rows by label (stable).  Each core owns 4 windows of 128
    consecutive sorted rows.  For window w starting at sorted position `base`,
    every positive of every row in the window lies within sorted positions
    [base-64, base+192)  (class size <= ~51 here; asserted).
  - Device computes, per window, the [128 x 256] Gram block
        G[p, j] = dot(x_row(base+p), x_col(base-64+j))
    via 4 accumulating fp8-e3m4 matmuls (K=512).  The lhsT operand is the
    center 128 columns of the same SBUF tile -- no separate lhs upload.
    The kernel is DMA-bound, so x is fp8 (e3m4 keeps 4 mantissa bits).
  - One DVE pass adds a host-built fp16 additive mask and a second pass
    row-max-reduces (tensor_tensor_reduce would fuse them but wedges TRN2):
        mask[p, j] = 256 - sq_col/2   if same label, not self, in range
                   = -30000           otherwise
        v[p] = max_j (G[p, j] + mask[p, j])
    The winner value encodes the hardest-positive distance:
        d2_ap = sq_row + 512 - 2*v        (recovered on host)
  - Everything else is exact host numpy: d_an from the raw fp32 batch,
    softplus tail, valid mask / count, final mean.  Device output is just
    [128, 4] fp32 row-max values per core.
  - DMAs alternate between the two hardware DGE queues (sync / scalar) in
    just-in-time order; the gpsimd software ring is avoided (slow).
"""

import os
import sys

import numpy as np

for _p in ("/opt/trn_rl_repo", "/root/.axon_site/_ro/trn_rl_repo"):
    if os.path.isdir(_p) and _p not in sys.path:
        sys.path.append(_p)

import ml_dtypes  # noqa: E402

import concourse.bass as bass  # noqa: E402
import concourse.bacc as bacc  # noqa: E402
import concourse.tile as tile  # noqa: E402
from concourse import mybir  # noqa: E402
from concourse import bass_utils  # noqa: E402

B = 4096
DIM = 512
C = 128
TEMP = 0.05
NCORES = 8
NW = 4            # windows of 128 sorted rows per core
W = 256           # columns per window (center 128 + 64 pad each side)
NK = DIM // 128   # 4 contraction tiles
LHS0 = 64         # offset of the window's own rows inside the 256 columns
NEG = -30000.0    # mask value for non-positive columns

F32 = mybir.dt.float32
F16 = mybir.dt.float16
E3M4 = mybir.dt.float8e3
ALU = mybir.AluOpType
AX = mybir.AxisListType

_NC_CACHE = None


def _build_nc():
    nc = bacc.Bacc(
        "TRN2",
        target_bir_lowering=False,
        debug=False,
        enable_asserts=False,
    )

    xw_d = nc.dram_tensor("xw", [128, NW, NK, W], E3M4, kind="ExternalInput").ap()
    mk_d = nc.dram_tensor("mk", [128, NW, W], F16, kind="ExternalInput").ap()
    out_d = nc.dram_tensor("out", [128, NW], F32, kind="ExternalOutput").ap()

    with tile.TileContext(nc) as tc:
        with (
            tc.tile_pool(name="big", bufs=1) as big,
            tc.tile_pool(name="ps", bufs=4, space="PSUM") as pp,
            tc.tile_pool(name="sm", bufs=1) as sm,
        ):
            # Two hardware DGE queues; alternate windows between them and
            # interleave x/mask just-in-time so neither ring backs up.
            engines = [nc.sync, nc.scalar]
            xt, mt = [], []
            for w in range(NW):
                t = big.tile([128, NK, W], E3M4, tag=f"x{w}", name=f"x{w}")
                xt.append(t)
                m = sm.tile([128, W], F16, tag=f"m{w}", name=f"m{w}")
                mt.append(m)
            for w in range(NW):
                engines[w % 2].dma_start(xt[w][:], xw_d[:, w])
                engines[w % 2].dma_start(mt[w][:], mk_d[:, w])

            outt = sm.tile([128, NW], F32, tag="outt")
            junk = sm.tile([128, W], F32, tag="junk")

            for w in range(NW):
                pt = pp.tile([128, W], F32, tag="acc", name="acc")
                for k in range(NK):
                    nc.tensor.matmul(
                        pt[:],
                        xt[w][:, k, LHS0:LHS0 + 128],
                        xt[w][:, k, :],
                        start=(k == 0),
                        stop=(k == NK - 1),
                    )
                # mask add + row max (tensor_tensor_reduce would fuse these
                # but wedges TRN2 hardware)
                nc.vector.tensor_tensor(
                    junk[:], pt[:], mt[w][:], op=ALU.add)
                nc.vector.tensor_reduce(
                    outt[:, w:w + 1], junk[:], axis=AX.X, op=ALU.max)

            nc.sync.dma_start(out_d[:], outt[:])

    nc.compile()
    return nc


def get_nc():
    global _NC_CACHE
    if _NC_CACHE is None:
        _NC_CACHE = _build_nc()
    return _NC_CACHE


def _prep_inputs(batch, labels, anchors=None, negatives=None):
    """Host-side prep: per-core window tensors + (order, sqs) for unshard."""
    batch = np.ascontiguousarray(np.asarray(batch), dtype=np.float32)
    labels = np.asarray(labels).astype(np.int64)

    order = np.argsort(labels, kind="stable").astype(np.int64)
    slab = labels[order]
    xs = batch[order]
    sqs = np.einsum("ij,ij->i", xs, xs, dtype=np.float64)

    xsT = np.ascontiguousarray(xs.T.astype(ml_dtypes.float8_e3m4))   # [DIM, B]
    maskvals = 256.0 - sqs / 2.0                                      # [B] f64

    # containment: every row's class fits in its window's 256 columns
    starts = np.searchsorted(slab, slab, side="left")
    ends = np.searchsorted(slab, slab, side="right")

    in_maps = []
    for c in range(NCORES):
        xw = np.empty((128, NW, NK, W), ml_dtypes.float8_e3m4)
        mk = np.empty((128, NW, W), np.float16)
        for wl in range(NW):
            base = (c * NW + wl) * 128
            assert starts[base] >= base - LHS0, "class overflows window left pad"
            assert ends[base + 127] <= base + (W - LHS0), (
                "class overflows window right pad")
            colpos = base - LHS0 + np.arange(W)
            validc = (colpos >= 0) & (colpos < B)
            cp = np.clip(colpos, 0, B - 1)
            xw[:, wl] = xsT[:, cp].reshape(NK, 128, W).transpose(1, 0, 2)
            rowpos = base + np.arange(128)
            ok = (validc[None, :]
                  & (slab[cp][None, :] == slab[rowpos][:, None])
                  & (colpos[None, :] != rowpos[:, None]))
            mk[:, wl] = np.where(ok, maskvals[cp][None, :], NEG)
        in_maps.append({"xw": xw, "mk": mk})
    return in_maps, order, sqs


def kernel(batch, labels, anchors=None, negatives=None, **_kwargs):
    batch = np.ascontiguousarray(np.asarray(batch), dtype=np.float32)
    labels_np = np.asarray(labels).astype(np.int64)
    negatives_np = np.asarray(negatives).astype(np.int64)

    in_maps, order, sqs = _prep_inputs(batch, labels_np)
    nc = get_nc()
    res = bass_utils.run_bass_kernel_spmd(nc, in_maps, core_ids=list(range(NCORES)))

    v = np.stack([np.asarray(r["out"], dtype=np.float64) for r in res.results])
    vsorted = v.transpose(0, 2, 1).reshape(B)     # [core, w, p] -> sorted pos
    d2ap_sorted = sqs + 512.0 - 2.0 * vsorted
    d2_ap = np.empty(B, dtype=np.float64)
    d2_ap[order] = d2ap_sorted
    d_ap = np.sqrt(np.maximum(d2_ap, 1e-12))

    diff = batch.astype(np.float64) - batch[negatives_np].astype(np.float64)
    d_an = np.sqrt(np.maximum(np.einsum("ij,ij->i", diff, diff), 1e-12))

    z = (d_ap - d_an) / (2.0 * TEMP)
    per = np.logaddexp(0.0, z)

    hist = np.bincount(labels_np, minlength=C)
    valid = (hist[labels_np] - 1) > 1
    count = float(valid.sum())
    loss = float((per * valid.astype(np.float64)).sum() / count)
    return np.array([loss], dtype=np.float32)
